# revision 53
# baseline (speedup 1.0000x reference)
"""Trainium2 Bass kernel for an autoregressive LSTM (warmup scan + decode).

Math (Keras LSTMCell, gate order i,f,g,o in the reference):
    z = x @ Wk + h @ Wr + b
    c = sigmoid(f)*c + sigmoid(i)*tanh(g)
    h = sigmoid(o)*tanh(c)
Warmup over T=256 input steps, then S=64 autoregressive decode steps through
a dense head p = h @ Wd + bd fed back as the next input.

Sharding: pure data-parallel over batch, 1024/8 = 128 examples per core
(128 = SBUF partition count). Weights replicated. No collectives.

Per-core layout: z is computed as [batch=128 part, 4096 gates] with the
batch-transposed activations as the matmul stationary operand and the
weights streaming, N=512 per PSUM bank. Gate columns are pre-permuted on the
host into NW=4 1024-wide "waves" [i_q|f_q|o_q|g_q] over unit-quarters; each
wave is a 2-bank PSUM tile (pool bufs=3) whose gate math starts while later
waves are still in the matmul stream. Within a wave the matmuls run k-outer
(x first, then h chunks) so the next step's PE work never waits on the
previous step's late h chunks. h is transposed back to [units, batch]
chunk-major layout with ONE merged DMA xbar transpose per wave (~1.25us
fixed cost regardless of size), off the compute engines.

fp8: the recurrent h @ Wr matmul — 94% of the MACs — runs in fp8-e4m3 with
perf_mode=DoubleRow (K=256 per stationary load) for ALL steps (warmup and
decode). Weights pre-scaled by SC=64 into e4m3's normal range; the gate
sigmoid undoes it with scale=1/SC. The g (candidate) columns carry an extra
2x so tanh(g)=2*sig(2g)-1 folds into ONE sigmoid ACT per wave (g-fold),
recovered by fused scalar_tensor_tensor DVE ops. The fp8 transposed state is
built by byte-interleaving wave pairs (w0,w1)/(w2,w3) and ONE 2-byte xbar
DMA transpose per pair. Decode keeps a bf16 hT (pair-merged transposes) for
the dense head only — an all-fp8 dense head measured rel err 0.032 > gate.

Scheduling (the big wins of this session, measured on NTFF traces):
- The steady state is LATENCY-bound, not PE-throughput-bound: the cycle is
  z1-close -> sigmoid -> DVE c-chain -> tanh(c) -> h-mul -> xbar transpose
  (~1.2us) -> DMA-completion sem (~1.2us) -> next step's first DR matmul.
- The PE MATMUL queue and ScalarE are strict FIFO, and the Tile scheduler's
  static order inflates the cycle; both are PINNED with sync=False deps
  (add_dep_helper): the PE order closes wave 1 mid-stream so its produce
  chain overlaps the remaining DR work, and the ScalarE order keeps
  tcc1/tcc0 ahead of sig2/sig3.
- Idempotent duplicate matmuls (start=True re-clears) fill the residual
  pair-0 wait so the HAM activity monitor never re-throttles the PE to
  1.2GHz mid-step (x-prefix dups in warmup, a wave-3 DR replay in decode).
Per-step cadence: warmup ~10.9us (PE floor ~9.4), decode ~14.6us.
"""

import sys

sys.path.insert(0, "/opt/trn_rl_repo")

import numpy as np

import concourse.bass as bass
import concourse.bacc as bacc
import concourse.mybir as mybir
from concourse.tile import TileContext, add_dep_helper
from concourse.bass_utils import run_bass_kernel_spmd

F32 = mybir.dt.float32
BF16 = mybir.dt.bfloat16
FP8 = mybir.dt.float8e4
NPBF16 = mybir.dt.np(mybir.dt.bfloat16)
NPFP8 = mybir.dt.np(mybir.dt.float8e4)
AF = mybir.ActivationFunctionType
DR = mybir.MatmulPerfMode.DoubleRow
SWI = mybir.MatmulPerfMode.DoubleRowSwInterleave

B, T, I, U, S = 1024, 256, 64, 1024, 64
NCORES = 8
BC = B // NCORES          # 128 batch per core
KX = I + 1                # x rows + ones row for folded bias
NU = U // 128             # 8 recurrent k-chunks (bf16)
NCH = NU // 2             # 4 DoubleRow k-chunks (fp8, K=256 each)
XBLK = 4                  # warmup steps per input-stream DMA block

NW = 4                    # waves per step (each covers U/NW units, 4U/NW z-cols)
QW = U // NW              # units per wave
WW = 4 * QW               # z columns per wave
NB = WW // 512            # PSUM banks (512-col matmuls) per wave

SC = 64.0                 # global weight scale: fp8 Wr lands in e4m3 normal range
SCI = 1.0 / SC

WARM_BF16_TAIL = 0        # all warmup in fp8
DECODE_FP8 = True         # decode recurrent + dense head in fp8


def _gate_perm():
    """Column permutation: reference gate order [i|f|g|o] (1024 each) ->
    NW waves of [i_q | f_q | o_q | g_q] (QW each)."""
    i0, f0, g0, o0 = 0, U, 2 * U, 3 * U
    parts = []
    for w in range(NW):
        for g in (i0, f0, o0, g0):
            parts.append(np.arange(QW) + g + w * QW)
    return np.concatenate(parts)


def _gate_colscale():
    """Per-column weight scale in permuted order: 2 for the g (candidate)
    columns, 1 elsewhere. tanh(g) = 2*sigmoid(2g) - 1, so doubling the g
    weight columns lets ONE sigmoid ACT cover the whole wave [i|f|o|g'] —
    one z-PSUM read instead of two frees the bank for the next step's
    x-matmuls a full ACT earlier, and drops 4 ACTs/step off ScalarE."""
    s = np.ones((NW, 4, QW), np.float32)
    s[:, 3, :] = 2.0
    return s.reshape(4 * U)


def build_nc(n_warm=T, n_dec=S - 1):
    nc = bacc.Bacc()

    n_steps = n_warm + n_dec

    def is_fp8(i):
        if i < n_warm:
            return i < n_warm - WARM_BF16_TAIL
        return DECODE_FP8

    nblk = (n_warm + XBLK - 1) // XBLK
    xTbD = nc.declare_dram_parameter("xTb", [nblk, KX, XBLK * BC], BF16, isOutput=False)
    WkD = nc.declare_dram_parameter("Wk", [KX, 4 * U], BF16, isOutput=False)
    WrD = nc.declare_dram_parameter("Wr", [128, NU, 4 * U], BF16, isOutput=False)
    Wr8D = nc.declare_dram_parameter("Wr8", [128, NCH, 2, 4 * U], FP8, isOutput=False)
    WdD = nc.declare_dram_parameter("Wd", [128, NU, I], BF16, isOutput=False)
    Wd8D = nc.declare_dram_parameter("Wd8", [128, NU, I], FP8, isOutput=False)
    bdD = nc.declare_dram_parameter("bdc", [I, 1], F32, isOutput=False)
    outD = nc.declare_dram_parameter("out", [n_dec + 1, I, BC], F32, isOutput=True)

    with TileContext(nc) as tc:
        with (
            tc.tile_pool(name="const", bufs=1) as cpool,
            tc.tile_pool(name="xp", bufs=2) as xpool,
            tc.tile_pool(name="state", bufs=3) as hpool,
            tc.tile_pool(name="state8", bufs=3) as hpool8,
            tc.tile_pool(name="gates", bufs=2) as gpool,
            tc.tile_pool(name="psum", bufs=4, space="PSUM") as zpool,
        ):
            Wk_sb = cpool.tile([KX, 4 * U], BF16)
            Wr_sb = cpool.tile([128, NU, 4 * U], BF16)
            Wr8_sb = cpool.tile([128, NCH, 2, 4 * U], FP8)
            Wd_sb = cpool.tile([128, NU, I], BF16)
            Wd8_sb = cpool.tile([128, NU, I], FP8)
            bd_sb = cpool.tile([I, 1], F32)
            c_sb = cpool.tile([128, U], F32)
            nc.sync.dma_start(Wk_sb[:], WkD[:])
            nc.sync.dma_start(Wr_sb[:], WrD[:])
            nc.sync.dma_start(Wr8_sb[:], Wr8D[:])
            nc.sync.dma_start(Wd_sb[:], WdD[:])
            nc.sync.dma_start(Wd8_sb[:], Wd8D[:])
            nc.sync.dma_start(bd_sb[:], bdD[:])
            nc.gpsimd.memset(c_sb[:], 0.0)

            nch = QW // 128   # 2 transposed 128-blocks per wave (= 1 DR chunk)

            act_insts = {}
            dve_insts = {}

            def pin_dve(order):
                """Pin the per-step DVE FIFO order: the scheduler runs the
                h-mul of wave 1 between t2(0) and c(0), delaying the c0 ->
                tanh(c0) -> pair-0 transpose chain by ~0.4us/step."""
                seq = [dve_insts[k] for k in order if k in dve_insts]
                for a, b in zip(seq, seq[1:]):
                    add_dep_helper(b.ins, a.ins, sync=False, reason="pinned DVE order")
                dve_insts.clear()

            def pin_scalar(order):
                """Pin the per-step ScalarE FIFO order (sync=False deps).
                The scheduler otherwise slots sig2 between tcc1 and tcc0,
                adding ~1.1us to the pair-0 produce chain, and the resulting
                >3.4us PE gap re-throttles HAM every step."""
                seq = [act_insts[k] for k in order if k in act_insts]
                for a, b in zip(seq, seq[1:]):
                    add_dep_helper(b.ins, a.ins, sync=False, reason="pinned ACT order")
                act_insts.clear()

            def gates_a(z, w):
                """Front half of wave w's gate math: one ACT + c update.

                g-fold: weights for the g columns are pre-scaled by 2, so ONE
                sigmoid over the whole wave [i|f|o|g'] yields sg = sig(2g)
                with tanh(g) = 2*sg - 1 recovered inside the fused DVE ops:
                  t2 = (sg - 0.5) * si        ( = tanh(g)*si / 2 )
                  c  = (t2 * 2) + f*c
                One PSUM read frees z's banks for the next step's x-matmuls
                as early as possible (the measured ~3us/step PE stall), and
                ScalarE drops from 12 to 8 ACTs/step.
                """
                sig = gpool.tile([128, 4 * QW], F32, tag="sig", name="sig")
                si = nc.scalar.activation(sig[:], z[:], AF.Sigmoid, scale=SCI)
                act_insts[("sig", w)] = si
                cw = c_sb[:, w * QW : (w + 1) * QW]
                t1 = gpool.tile([128, QW], F32, tag="t1", name="t1")
                t2 = gpool.tile([128, QW], F32, tag="t2", name="t2")
                dve_insts[("t1", w)] = nc.vector.tensor_mul(
                    t1[:], sig[:, QW : 2 * QW], cw
                )
                dve_insts[("t2", w)] = nc.vector.scalar_tensor_tensor(
                    t2[:], sig[:, 3 * QW :], 0.5, sig[:, 0:QW],
                    op0=mybir.AluOpType.subtract, op1=mybir.AluOpType.mult,
                )
                dve_insts[("c", w)] = nc.vector.scalar_tensor_tensor(
                    cw, t2[:], 2.0, t1[:],
                    op0=mybir.AluOpType.mult, op1=mybir.AluOpType.add,
                )
                return sig

            def gates_b(sig, w, hT_new, hT8_new):
                """Back half: tanh(c), h, transpose.

                bf16 mode (hT_new): h -> bf16 tile -> per-wave xbar transpose.
                fp8 mode (hT8_new): h is written as fp8 directly by the DVE
                mul, byte-interleaved with the partner wave of its pair
                (w0,w1)/(w2,w3); ONE 2-byte xbar transpose per pair then
                yields the DoubleRow stationary layout in place — no cast,
                no gpsimd, half the transposes. (Verified on HW: DR accepts
                lhsT APs with Ko stride 256B / M stride 2B.)
                """
                cw = c_sb[:, w * QW : (w + 1) * QW]
                tcc = gpool.tile([128, QW], F32, tag="tcc", name="tcc")
                act_insts[("tcc", w)] = nc.scalar.activation(tcc[:], cw, AF.Tanh)
                if hT8_new is not None:
                    k, s = PAIRK[w], SLOT[w]
                    if s == 0:
                        pair_state[k] = gpool.tile(
                            [128, QW, 2], FP8, tag="hp8", name="hp8"
                        )
                    hp8 = pair_state[k]
                    dve_insts[("h", w)] = nc.vector.tensor_mul(
                        hp8[:, :, s], sig[:, 2 * QW : 3 * QW], tcc[:]
                    )
                    if s == 1:
                        nc.sync.dma_start_transpose(
                            hT8_new[:, k, :, :], hp8[:].bitcast(BF16)
                        )
                        del pair_state[k]
                if hT_new is not None:
                    # Pair-merged bf16 transpose: ONE DMAT per wave pair
                    # (w0,w1)/(w2,w3) instead of one per wave — the xbar
                    # serializes transposes at ~1.3us each, and decode issues
                    # fp8-pair + bf16 transposes back-to-back. Slot = w%2 so
                    # the container is unit-ascending -> chunk-major dst.
                    k, sb = PAIRK[w], w % 2
                    if k not in pairb_state:
                        pairb_state[k] = (
                            gpool.tile([128, 2, QW], BF16, tag="hbf", name="hbf"),
                            set(),
                        )
                    hbf, written = pairb_state[k]
                    dve_insts[("hb", w)] = nc.vector.tensor_mul(
                        hbf[:, sb, :], sig[:, 2 * QW : 3 * QW], tcc[:]
                    )
                    written.add(w)
                    if len(written) == 2:
                        nc.sync.dma_start_transpose(
                            hT_new[:, 4 * k : 4 * (k + 1), :], hbf[:]
                        )
                        del pairb_state[k]

            def emit_gates(z, w, hT_new, hT8_new, merged=False):
                """Unsplit gate math (bf16 steps): gates_a + gates_b."""
                sig = gates_a(z, w)
                gates_b(sig, w, hT_new, hT8_new)

            # fp8 pair-pack layout: wave w's h lands in pair k = PAIRK[w] at
            # byte slot SLOT[w]; the pair is transposed after its second
            # writer in gate-B order [1, 0, 2, 3].
            PAIRK = {0: 0, 1: 0, 2: 1, 3: 1}
            SLOT = {1: 0, 0: 1, 2: 0, 3: 1}
            pair_state = {}
            pairb_state = {}

            def h8chunk(hT8, ch):
                """DoubleRow lhsT AP [128, 2, 128] for chunk ch (units of
                wave ch) out of the pair-transposed container."""
                v = hT8[:, PAIRK[ch], :, :].bitcast(FP8).rearrange(
                    "p a (m s) -> p a m s", s=2
                )
                return v[:, :, :, SLOT[ch]]

            def emit_step_fp8(x_lhsT, hT8_prev, fp8_out, bf16_also=False):
                """One fp8 LSTM step (DoubleRow h matmuls).

                The PE order is PINNED with sync=False deps (the Tile
                scheduler otherwise reorders all ch2/ch3 sections to the
                stream tail, which pushes z1's close — and with it the whole
                sig->DVE->tanh->h->transpose produce chain — 2.4us later,
                inflating the steady-state period). Pinning closes wave 1 at
                ~MM 16 of 40: the pair-0 chain starts mid-stream and the
                next step's first DR consumption meets it with ~0 idle.
                """
                hT_new = (
                    hpool.tile([128, NU, 128], BF16, tag="hT", name="hT_new")
                    if (bf16_also or not fp8_out)
                    else None
                )
                hT8_new = (
                    hpool8.tile([128, 2, 2, 128], BF16, tag="hT8", name="hT8_new")
                    if fp8_out
                    else None
                )
                zs = [
                    zpool.tile([128, WW], F32, tag="z", name=f"z{w}") for w in range(NW)
                ]
                x_only = hT8_prev is None
                pe = []

                def xsec(w, stop=False):
                    for n in range(NB):
                        pe.append(nc.tensor.matmul(
                            zs[w][:, n * 512 : (n + 1) * 512],
                            x_lhsT,
                            Wk_sb[:, WW * w + n * 512 : WW * w + (n + 1) * 512],
                            start=True,
                            stop=stop,
                        ))

                A = {}
                def close(w):
                    A[w] = gates_a(zs[w], w)
                def flush(w):
                    gates_b(A.pop(w), w, hT_new, hT8_new)

                if x_only:
                    for w in (1, 0, 2, 3):
                        xsec(w, stop=True)
                    close(1); close(0); flush(1); flush(0)
                    close(2); close(3); flush(2); flush(3)
                    return hT_new, hT8_new

                def dr(w, ch, stop=False):
                    for n in range(NB):
                        pe.append(nc.tensor.matmul(
                            zs[w][:, n * 512 : (n + 1) * 512],
                            h8chunk(hT8_prev, ch),
                            Wr8_sb[:, ch, :, WW * w + n * 512 : WW * w + (n + 1) * 512],
                            start=False,
                            stop=stop,
                            perf_mode=DR,
                        ))

                xsec(1); xsec(0)
                # PE-warming filler: duplicate x-matmuls (start=True re-clears
                # and rewrites the same contribution — idempotent since the DR
                # accumulates come later). The ~2.5us pair-0 wait otherwise
                # re-throttles HAM to 1.2GHz, making the first ~2.6us of DR
                # matmuls run at half clock every step.
                for _ in range(3):
                    xsec(1); xsec(0)   # PE-warming duplicate rounds
                dr(1, 0); dr(1, 1); dr(0, 0); dr(0, 1)
                xsec(2); dr(2, 0); dr(2, 1)
                dr(1, 2); dr(1, 3, stop=True); close(1)
                dr(0, 2); dr(0, 3, stop=True); close(0)
                xsec(3); dr(3, 0); dr(3, 1)
                dr(2, 2); dr(2, 3, stop=True); close(2)
                flush(1); flush(0)
                dr(3, 2); dr(3, 3, stop=True); close(3)
                flush(2); flush(3)
                for a, b in zip(pe, pe[1:]):
                    add_dep_helper(b.ins, a.ins, sync=False, reason="pinned PE order")
                pin_scalar([("sig", 1), ("sig", 0), ("tcc", 1), ("tcc", 0),
                            ("sig", 2), ("sig", 3), ("tcc", 2), ("tcc", 3)])
                pin_dve([("t1", 1), ("t2", 1), ("c", 1),
                         ("t1", 0), ("t2", 0), ("c", 0),
                         ("h", 1), ("hb", 1), ("h", 0), ("hb", 0),
                         ("t1", 2), ("t2", 2), ("c", 2),
                         ("t1", 3), ("t2", 3), ("c", 3),
                         ("h", 2), ("hb", 2), ("h", 3), ("hb", 3)])
                return hT_new, hT8_new

            def emit_step(x_lhsT, hT_prev, x_first):
                """One bf16 LSTM step; returns (hT_new bf16, None)."""
                hT_new = hpool.tile([128, NU, 128], BF16, tag="hT", name="hT_new")
                hT8_new = None
                for w in range(NW):
                    base = WW * w
                    z = zpool.tile([128, WW], F32, tag="z", name="z")
                    ks = []
                    if x_first:
                        ks.append(("x", 0))
                    if hT_prev is not None:
                        ks += [("h", u) for u in range(NU)]
                    if not x_first:
                        ks.append(("x", 0))
                    # k-outer / n-inner: the first-emitted matmuls depend on
                    # operands ready earliest (x, then low h chunks), so the
                    # PE can start the next step while the previous step's
                    # late h chunks are still in flight through the
                    # gate-math chain.
                    for ki, (kind, kv) in enumerate(ks):
                        st, sp = ki == 0, ki == len(ks) - 1
                        for n in range(NB):
                            zsl = z[:, n * 512 : (n + 1) * 512]
                            cs = slice(base + n * 512, base + (n + 1) * 512)
                            if kind == "x":
                                nc.tensor.matmul(
                                    zsl, x_lhsT, Wk_sb[:, cs], start=st, stop=sp
                                )
                            else:
                                nc.tensor.matmul(
                                    zsl,
                                    hT_prev[:, kv, :],
                                    Wr_sb[:, kv, cs],
                                    start=st,
                                    stop=sp,
                                )
                    emit_gates(z, w, hT_new, hT8_new, merged=False)
                return hT_new, hT8_new

            def emit_dense(hT_cur, out_idx, feedback, zp=None, pe=None):
                if zp is None:
                    zp = zpool.tile([128, WW], F32, tag="z", name="zdense")
                pp = zp[0:I, 0:BC]
                for u in range(NU):
                    mm = nc.tensor.matmul(
                        pp,
                        Wd_sb[:, u, :],
                        hT_cur[:, u, :],
                        start=(u == 0),
                        stop=(u == NU - 1),
                    )
                    if pe is not None:
                        pe.append(mm)
                if feedback:
                    pt = gpool.tile([KX, BC], BF16, tag="pT", name="pT")
                    nc.gpsimd.memset(pt[I : I + 1, :], 1.0)
                    act_insts[("pt", 0)] = nc.scalar.activation(
                        pt[0:I, :], pp, AF.Identity, bias=bd_sb[:]
                    )
                else:
                    pt = None
                pf = gpool.tile([I, BC], F32, tag="pf", name="pf")
                act_insts[("pf", 0)] = nc.scalar.activation(
                    pf[:], pp, AF.Identity, bias=bd_sb[:]
                )
                nc.scalar.dma_start(outD[out_idx], pf[:])
                return pt

            def emit_step_dec(hT_prev, out_idx):
                """Decode step: consumes hT_prev for BOTH the recurrent
                matmuls and the dense head of the PREVIOUS step's output
                (out_idx), whose result pt feeds this step's x-part. The
                dense matmuls run mid-stream (after wave 1's h-section,
                when hT_prev's late chunks have landed) and all x-matmuls
                come after pt, so the PE never idles waiting for the
                h-transpose tail + dense chain between steps."""
                hT_new = hpool.tile([128, NU, 128], BF16, tag="hT", name="hT_new")
                zp = zpool.tile([128, WW], F32, tag="z", name="zdense")
                zs = [
                    zpool.tile([128, WW], F32, tag="z", name=f"z{w}") for w in range(NW)
                ]

                def hsec(w):
                    for u in range(NU):
                        for n in range(NB):
                            nc.tensor.matmul(
                                zs[w][:, n * 512 : (n + 1) * 512],
                                hT_prev[:, u, :],
                                Wr_sb[:, u, WW * w + n * 512 : WW * w + (n + 1) * 512],
                                start=(u == 0),
                                stop=False,
                            )

                def xsec(w):
                    for n in range(NB):
                        nc.tensor.matmul(
                            zs[w][:, n * 512 : (n + 1) * 512],
                            pt[:],
                            Wk_sb[:, WW * w + n * 512 : WW * w + (n + 1) * 512],
                            start=False,
                            stop=True,
                        )

                hsec(0)
                hsec(1)
                pt = emit_dense(hT_prev, out_idx, feedback=True, zp=zp)
                xsec(0)
                emit_gates(zs[0], 0, hT_new, None)
                xsec(1)
                emit_gates(zs[1], 1, hT_new, None)
                for w in (2, 3):
                    hsec(w)
                    xsec(w)
                    emit_gates(zs[w], w, hT_new, None)
                return hT_new

            def emit_step_dec8(hT_prev, hT8_prev, out_idx):
                """fp8 decode step: DR recurrent matmuls (fp8 pairs) + bf16
                dense head (reads hT_prev; fp8 h x fp8 Wd straight to the
                output measured rel-err 0.032 > gate) of the PREVIOUS step's
                output (out_idx); pt feeds this step's x-part, which closes
                each wave (start comes from dr(w,0))."""
                hT_new = hpool.tile([128, NU, 128], BF16, tag="hT", name="hT_new")
                hT8_new = hpool8.tile([128, 2, 2, 128], BF16, tag="hT8", name="hT8_new")
                # No 5th PSUM tile for the dense head (4 z tiles = all 8
                # banks): pp borrows zs[3]'s bank n0. dr(3,0)'s start=True
                # reclears it after the pt/pf reads (WAR dep), so wave 3's
                # sections are emitted after the dense.
                zs = [
                    zpool.tile([128, WW], F32, tag="z", name=f"z{w}") for w in range(NW)
                ]

                pe = []

                def dr(w, ch, start=False, stop=False):
                    for n in range(NB):
                        pe.append(nc.tensor.matmul(
                            zs[w][:, n * 512 : (n + 1) * 512],
                            h8chunk(hT8_prev, ch),
                            Wr8_sb[:, ch, :, WW * w + n * 512 : WW * w + (n + 1) * 512],
                            start=start,
                            stop=stop,
                            perf_mode=DR,
                        ))

                def xse(w):
                    for n in range(NB):
                        pe.append(nc.tensor.matmul(
                            zs[w][:, n * 512 : (n + 1) * 512],
                            pt[:],
                            Wk_sb[:, WW * w + n * 512 : WW * w + (n + 1) * 512],
                            start=False,
                            stop=True,
                        ))

                A = {}
                def close(w):
                    A[w] = gates_a(zs[w], w)
                def flush(w):
                    gates_b(A.pop(w), w, hT_new, hT8_new)

                # Pinned PE order (see emit_step_fp8): pair0 chunks of waves
                # 1,0,2 first, then pair1 chunks of w1, then dense (needs all
                # of hT_prev) + pt, then x closes each wave; wave 3 entirely
                # after the dense (its banks host pp).
                dr(1, 0, start=True); dr(1, 1); dr(0, 0, start=True); dr(0, 1)
                dr(2, 0, start=True); dr(2, 1)
                dr(1, 2); dr(1, 3)
                pt = emit_dense(hT_prev, out_idx, feedback=True, zp=zs[3],
                                pe=pe)
                xse(1); close(1)
                dr(0, 2); dr(0, 3); xse(0); close(0)
                dr(2, 2); dr(2, 3); xse(2); close(2)
                flush(1); flush(0)
                dr(3, 0, start=True); dr(3, 1); dr(3, 2); dr(3, 3); xse(3)
                close(3); flush(2); flush(3)
                # PE-warming filler: replay wave 3's DR sections into z3 after
                # sig3's read (WAR-ordered; nothing reads the replay — the
                # next step's dr(3,0) start=True reclears). Covers most of the
                # ~3.2us pair-0 wait so HAM stays at full clock.
                dr(3, 0, start=True); dr(3, 1); dr(3, 2); dr(3, 3, stop=True)
                dr(2, 0, start=True); dr(2, 1); dr(2, 2); dr(2, 3, stop=True)
                for a, b in zip(pe, pe[1:]):
                    add_dep_helper(b.ins, a.ins, sync=False, reason="pinned PE order")
                pin_scalar([("pt", 0), ("sig", 1), ("sig", 0), ("pf", 0),
                            ("tcc", 1), ("tcc", 0), ("sig", 2), ("sig", 3),
                            ("tcc", 2), ("tcc", 3)])
                pin_dve([("t1", 1), ("t2", 1), ("c", 1),
                         ("t1", 0), ("t2", 0), ("c", 0),
                         ("h", 1), ("hb", 1), ("h", 0), ("hb", 0),
                         ("t1", 2), ("t2", 2), ("c", 2),
                         ("t1", 3), ("t2", 3), ("c", 3),
                         ("h", 2), ("hb", 2), ("h", 3), ("hb", 3)])
                return hT_new, hT8_new

            hT = hT8 = None
            # prefetch input blocks one block (XBLK steps) ahead so the
            # stream DMA never sits on the first x-matmul's critical path
            nblk_used = (n_warm + XBLK - 1) // XBLK
            xtiles = {}
            if nblk_used > 0:
                xtiles[0] = xpool.tile([KX, XBLK * BC], BF16, tag="xblk", name="xblk")
                nc.sync.dma_start(xtiles[0][:], xTbD[0])
            for t in range(n_warm):
                b = t // XBLK
                s = t % XBLK
                f_out = is_fp8(t + 1) if t + 1 < n_steps else False
                xl = xtiles[b][:, s * BC : (s + 1) * BC]
                if is_fp8(t):
                    both = DECODE_FP8 and n_dec > 0 and t == n_warm - 1
                    hT, hT8 = emit_step_fp8(xl, hT8, f_out, bf16_also=both)
                else:
                    hT, hT8 = emit_step(xl, hT, x_first=True)
                if t % XBLK == 0 and b + 1 < nblk_used:
                    # prefetch the next input block; emitted after the step so
                    # it queues behind this step's critical hT transposes
                    xtiles[b + 1] = xpool.tile([KX, XBLK * BC], BF16, tag="xblk", name="xblk")
                    nc.sync.dma_start(xtiles[b + 1][:], xTbD[b + 1])
                xtiles.pop(b - 1, None)
            if n_dec == 0:
                emit_dense(hT, 0, feedback=False)
            elif DECODE_FP8:
                for s_ in range(1, n_dec + 1):
                    hT, hT8 = emit_step_dec8(hT, hT8, s_ - 1)
                emit_dense(hT, n_dec, feedback=False)
            else:
                for s_ in range(1, n_dec + 1):
                    hT = emit_step_dec(hT, s_ - 1)
                emit_dense(hT, n_dec, feedback=False)

    nc.finalize()
    return nc


def prep_in_maps(inputs, Wk, Wr, b, Wd, bd, n_warm=T):
    """Host-side sharding + layout. inputs [B, T, I] fp32; returns 8 in_maps."""
    perm = _gate_perm()
    cs = SC * _gate_colscale()                                 # 2*SC on g columns
    Wk_aug = np.concatenate(
        [np.asarray(Wk, np.float32), np.asarray(b, np.float32)[None, :]], axis=0
    )
    Wk_p = (Wk_aug[:, perm] * cs).astype(NPBF16)               # [65, 4096] scaled
    Wr_s = np.asarray(Wr, np.float32)[:, perm] * cs            # [1024, 4096] scaled
    Wr_p = Wr_s.reshape(NU, 128, 4 * U).transpose(1, 0, 2).astype(NPBF16).copy()
    # DoubleRow layout: [p, ch, j, n] = Wr_s[ch*256 + j*128 + p, n]
    Wr_8 = (
        Wr_s.reshape(NCH, 2, 128, 4 * U).transpose(2, 0, 1, 3).astype(NPFP8).copy()
    )
    Wd_f = np.asarray(Wd, np.float32).reshape(NU, 128, I).transpose(1, 0, 2)
    Wd_p = Wd_f.astype(NPBF16).copy()
    Wd_8 = (Wd_f * SC).astype(NPFP8).copy()
    bd_c = np.asarray(bd, np.float32).reshape(I, 1).copy()

    x = np.asarray(inputs, np.float32)
    nblk = (n_warm + XBLK - 1) // XBLK
    in_maps = []
    for c in range(NCORES):
        xc = x[c * BC : (c + 1) * BC, :n_warm]                 # [BC, n_warm, I]
        xT = np.transpose(xc, (1, 2, 0))                       # [n_warm, I, BC]
        xTa = np.concatenate([xT, np.ones((n_warm, 1, BC), np.float32)], axis=1)
        if nblk * XBLK != n_warm:
            pad = np.zeros((nblk * XBLK - n_warm, KX, BC), np.float32)
            xTa = np.concatenate([xTa, pad], axis=0)
        xTb = (
            xTa.reshape(nblk, XBLK, KX, BC)
            .transpose(0, 2, 1, 3)
            .reshape(nblk, KX, XBLK * BC)
            .astype(NPBF16)
            .copy()
        )
        in_maps.append(
            {"xTb": xTb, "Wk": Wk_p, "Wr": Wr_p, "Wr8": Wr_8, "Wd": Wd_p,
             "Wd8": Wd_8, "bdc": bd_c}
        )
    return in_maps


_NC_CACHE = {}


def _get_nc(n_warm, n_dec):
    key = (n_warm, n_dec)
    if key not in _NC_CACHE:
        _NC_CACHE[key] = build_nc(n_warm, n_dec)
    return _NC_CACHE[key]


def run(inputs, Wk, Wr, b, Wd, bd, n_warm, n_dec, trace=False, **kw):
    nc = _get_nc(n_warm, n_dec)
    in_maps = prep_in_maps(inputs, Wk, Wr, b, Wd, bd, n_warm)
    res = run_bass_kernel_spmd(nc, in_maps, list(range(NCORES)), trace=trace, **kw)
    outs = [np.asarray(res.results[c]["out"], np.float32) for c in range(NCORES)]
    # out[c]: [n_dec+1, I, BC] -> preds [B, n_dec+1, I]
    preds = np.concatenate([o.transpose(2, 0, 1) for o in outs], axis=0)
    return preds, res


def kernel(inputs, Wk, Wr, b, Wd, bd, output_indices, output_steps):
    n_dec = int(output_steps) - 1
    preds, _ = run(inputs, Wk, Wr, b, Wd, bd, T, n_dec)
    idx = np.asarray(output_indices, np.int64)
    return np.take(preds, idx, axis=-1).astype(np.float32)



# revision 56
# speedup vs baseline: 1.0345x; 1.0345x over previous
"""Trainium2 Bass kernel for an autoregressive LSTM (warmup scan + decode).

Math (Keras LSTMCell, gate order i,f,g,o in the reference):
    z = x @ Wk + h @ Wr + b
    c = sigmoid(f)*c + sigmoid(i)*tanh(g)
    h = sigmoid(o)*tanh(c)
Warmup over T=256 input steps, then S=64 autoregressive decode steps through
a dense head p = h @ Wd + bd fed back as the next input.

Sharding: pure data-parallel over batch, 1024/8 = 128 examples per core
(128 = SBUF partition count). Weights replicated. No collectives.

Per-core layout: z is computed as [batch=128 part, 4096 gates] with the
batch-transposed activations as the matmul stationary operand and the
weights streaming, N=512 per PSUM bank. Gate columns are pre-permuted on the
host into NW=4 1024-wide "waves" [i_q|f_q|o_q|g_q] over unit-quarters; each
wave is a 2-bank PSUM tile (pool bufs=3) whose gate math starts while later
waves are still in the matmul stream. Within a wave the matmuls run k-outer
(x first, then h chunks) so the next step's PE work never waits on the
previous step's late h chunks. h is transposed back to [units, batch]
chunk-major layout with ONE merged DMA xbar transpose per wave (~1.25us
fixed cost regardless of size), off the compute engines.

fp8: the recurrent h @ Wr matmul — 94% of the MACs — runs in fp8-e4m3 with
perf_mode=DoubleRow (K=256 per stationary load) for ALL steps (warmup and
decode). Weights pre-scaled by SC=64 into e4m3's normal range; the gate
sigmoid undoes it with scale=1/SC. The g (candidate) columns carry an extra
2x so tanh(g)=2*sig(2g)-1 folds into ONE sigmoid ACT per wave (g-fold),
recovered by fused scalar_tensor_tensor DVE ops. The fp8 transposed state is
built by byte-interleaving wave pairs (w0,w1)/(w2,w3) and ONE 2-byte xbar
DMA transpose per pair. Decode keeps a bf16 hT (pair-merged transposes) for
the dense head only — an all-fp8 dense head measured rel err 0.032 > gate.

Scheduling (the big wins of this session, measured on NTFF traces):
- The steady state is LATENCY-bound, not PE-throughput-bound: the cycle is
  z1-close -> sigmoid -> DVE c-chain -> tanh(c) -> h-mul -> xbar transpose
  (~1.2us) -> DMA-completion sem (~1.2us) -> next step's first DR matmul.
- The PE MATMUL queue and ScalarE are strict FIFO, and the Tile scheduler's
  static order inflates the cycle; both are PINNED with sync=False deps
  (add_dep_helper): the PE order closes wave 1 mid-stream so its produce
  chain overlaps the remaining DR work, and the ScalarE order keeps
  tcc1/tcc0 ahead of sig2/sig3.
- Idempotent duplicate matmuls (start=True re-clears) fill the residual
  pair-0 wait so the HAM activity monitor never re-throttles the PE to
  1.2GHz mid-step (x-prefix dups in warmup, a wave-3 DR replay in decode).
Per-step cadence: warmup ~10.9us (PE floor ~9.4), decode ~14.6us.
"""

import sys

sys.path.insert(0, "/opt/trn_rl_repo")

import numpy as np

import concourse.bass as bass
import concourse.bacc as bacc
import concourse.mybir as mybir
from concourse.tile import TileContext, add_dep_helper
from concourse.bass_utils import run_bass_kernel_spmd

F32 = mybir.dt.float32
BF16 = mybir.dt.bfloat16
FP8 = mybir.dt.float8e4
NPBF16 = mybir.dt.np(mybir.dt.bfloat16)
NPFP8 = mybir.dt.np(mybir.dt.float8e4)
AF = mybir.ActivationFunctionType
DR = mybir.MatmulPerfMode.DoubleRow
SWI = mybir.MatmulPerfMode.DoubleRowSwInterleave

B, T, I, U, S = 1024, 256, 64, 1024, 64
NCORES = 8
BC = B // NCORES          # 128 batch per core
KX = I + 1                # x rows + ones row for folded bias
NU = U // 128             # 8 recurrent k-chunks (bf16)
NCH = NU // 2             # 4 DoubleRow k-chunks (fp8, K=256 each)
XBLK = 4                  # warmup steps per input-stream DMA block

NW = 4                    # waves per step (each covers U/NW units, 4U/NW z-cols)
QW = U // NW              # units per wave
WW = 4 * QW               # z columns per wave
NB = WW // 512            # PSUM banks (512-col matmuls) per wave

SC = 64.0                 # global weight scale: fp8 Wr lands in e4m3 normal range
SCI = 1.0 / SC

WARM_BF16_TAIL = 0        # all warmup in fp8
DECODE_FP8 = True         # decode recurrent + dense head in fp8


def _gate_perm():
    """Column permutation: reference gate order [i|f|g|o] (1024 each) ->
    NW waves of [i_q | f_q | o_q | g_q] (QW each)."""
    i0, f0, g0, o0 = 0, U, 2 * U, 3 * U
    parts = []
    for w in range(NW):
        for g in (i0, f0, o0, g0):
            parts.append(np.arange(QW) + g + w * QW)
    return np.concatenate(parts)


def _gate_colscale():
    """Per-column weight scale in permuted order: 2 for the g (candidate)
    columns, 1 elsewhere. tanh(g) = 2*sigmoid(2g) - 1, so doubling the g
    weight columns lets ONE sigmoid ACT cover the whole wave [i|f|o|g'] —
    one z-PSUM read instead of two frees the bank for the next step's
    x-matmuls a full ACT earlier, and drops 4 ACTs/step off ScalarE."""
    s = np.ones((NW, 4, QW), np.float32)
    s[:, 3, :] = 2.0
    return s.reshape(4 * U)


def build_nc(n_warm=T, n_dec=S - 1):
    nc = bacc.Bacc()

    n_steps = n_warm + n_dec

    def is_fp8(i):
        if i < n_warm:
            return i < n_warm - WARM_BF16_TAIL
        return DECODE_FP8

    nblk = (n_warm + XBLK - 1) // XBLK
    xTbD = nc.declare_dram_parameter("xTb", [nblk, KX, XBLK * BC], BF16, isOutput=False)
    WkD = nc.declare_dram_parameter("Wk", [KX, 4 * U], BF16, isOutput=False)
    WrD = nc.declare_dram_parameter("Wr", [128, NU, 4 * U], BF16, isOutput=False)
    Wr8D = nc.declare_dram_parameter("Wr8", [128, NCH, 2, 4 * U], FP8, isOutput=False)
    WdD = nc.declare_dram_parameter("Wd", [128, NU, I], BF16, isOutput=False)
    Wd8D = nc.declare_dram_parameter("Wd8", [128, NU, I], FP8, isOutput=False)
    bdD = nc.declare_dram_parameter("bdc", [I, 1], F32, isOutput=False)
    outD = nc.declare_dram_parameter("out", [n_dec + 1, I, BC], F32, isOutput=True)

    with TileContext(nc) as tc:
        with (
            tc.tile_pool(name="const", bufs=1) as cpool,
            tc.tile_pool(name="xp", bufs=2) as xpool,
            tc.tile_pool(name="state", bufs=3) as hpool,
            tc.tile_pool(name="state8", bufs=3) as hpool8,
            tc.tile_pool(name="gates", bufs=2) as gpool,
            tc.tile_pool(name="psum", bufs=4, space="PSUM") as zpool,
        ):
            Wk_sb = cpool.tile([KX, 4 * U], BF16)
            Wr_sb = cpool.tile([128, NU, 4 * U], BF16)
            Wr8_sb = cpool.tile([128, NCH, 2, 4 * U], FP8)
            Wd_sb = cpool.tile([128, NU, I], BF16)
            Wd8_sb = cpool.tile([128, NU, I], FP8)
            bd_sb = cpool.tile([I, 1], F32)
            c_sb = cpool.tile([128, U], F32)
            nc.sync.dma_start(Wk_sb[:], WkD[:])
            nc.sync.dma_start(Wr_sb[:], WrD[:])
            nc.sync.dma_start(Wr8_sb[:], Wr8D[:])
            nc.sync.dma_start(Wd_sb[:], WdD[:])
            nc.sync.dma_start(Wd8_sb[:], Wd8D[:])
            nc.sync.dma_start(bd_sb[:], bdD[:])
            nc.gpsimd.memset(c_sb[:], 0.0)

            nch = QW // 128   # 2 transposed 128-blocks per wave (= 1 DR chunk)

            act_insts = {}
            dve_insts = {}

            def pin_dve(order):
                """Pin the per-step DVE FIFO order: the scheduler runs the
                h-mul of wave 1 between t2(0) and c(0), delaying the c0 ->
                tanh(c0) -> pair-0 transpose chain by ~0.4us/step."""
                seq = [dve_insts[k] for k in order if k in dve_insts]
                for a, b in zip(seq, seq[1:]):
                    add_dep_helper(b.ins, a.ins, sync=False, reason="pinned DVE order")
                dve_insts.clear()

            def pin_scalar(order):
                """Pin the per-step ScalarE FIFO order (sync=False deps).
                The scheduler otherwise slots sig2 between tcc1 and tcc0,
                adding ~1.1us to the pair-0 produce chain, and the resulting
                >3.4us PE gap re-throttles HAM every step."""
                seq = [act_insts[k] for k in order if k in act_insts]
                for a, b in zip(seq, seq[1:]):
                    add_dep_helper(b.ins, a.ins, sync=False, reason="pinned ACT order")
                act_insts.clear()

            def gates_a(z, w):
                """Front half of wave w's gate math: one ACT + c update.

                g-fold: weights for the g columns are pre-scaled by 2, so ONE
                sigmoid over the whole wave [i|f|o|g'] yields sg = sig(2g)
                with tanh(g) = 2*sg - 1 recovered inside the fused DVE ops:
                  t2 = (sg - 0.5) * si        ( = tanh(g)*si / 2 )
                  c  = (t2 * 2) + f*c
                One PSUM read frees z's banks for the next step's x-matmuls
                as early as possible (the measured ~3us/step PE stall), and
                ScalarE drops from 12 to 8 ACTs/step.
                """
                sig = gpool.tile([128, 4 * QW], F32, tag="sig", name="sig")
                si = nc.scalar.activation(sig[:], z[:], AF.Sigmoid, scale=SCI)
                act_insts[("sig", w)] = si
                cw = c_sb[:, w * QW : (w + 1) * QW]
                t1 = gpool.tile([128, QW], F32, tag="t1", name="t1")
                t2 = gpool.tile([128, QW], F32, tag="t2", name="t2")
                dve_insts[("t1", w)] = nc.vector.tensor_mul(
                    t1[:], sig[:, QW : 2 * QW], cw
                )
                dve_insts[("t2", w)] = nc.vector.scalar_tensor_tensor(
                    t2[:], sig[:, 3 * QW :], 0.5, sig[:, 0:QW],
                    op0=mybir.AluOpType.subtract, op1=mybir.AluOpType.mult,
                )
                dve_insts[("c", w)] = nc.vector.scalar_tensor_tensor(
                    cw, t2[:], 2.0, t1[:],
                    op0=mybir.AluOpType.mult, op1=mybir.AluOpType.add,
                )
                return sig

            def gates_b(sig, w, hT_new, hT8_new):
                """Back half: tanh(c), h, transpose.

                bf16 mode (hT_new): h -> bf16 tile -> per-wave xbar transpose.
                fp8 mode (hT8_new): h is written as fp8 directly by the DVE
                mul, byte-interleaved with the partner wave of its pair
                (w0,w1)/(w2,w3); ONE 2-byte xbar transpose per pair then
                yields the DoubleRow stationary layout in place — no cast,
                no gpsimd, half the transposes. (Verified on HW: DR accepts
                lhsT APs with Ko stride 256B / M stride 2B.)
                """
                cw = c_sb[:, w * QW : (w + 1) * QW]
                tcc = gpool.tile([128, QW], F32, tag="tcc", name="tcc")
                act_insts[("tcc", w)] = nc.scalar.activation(tcc[:], cw, AF.Tanh)
                fp8_dmat = None
                if hT8_new is not None:
                    k, s = PAIRK[w], SLOT[w]
                    if s == 0:
                        pair_state[k] = gpool.tile(
                            [128, QW, 2], FP8, tag="hp8", name="hp8"
                        )
                    hp8 = pair_state[k]
                    dve_insts[("h", w)] = nc.vector.tensor_mul(
                        hp8[:, :, s], sig[:, 2 * QW : 3 * QW], tcc[:]
                    )
                    if s == 1:
                        def fp8_dmat(k=k, hp8=hp8):
                            nc.sync.dma_start_transpose(
                                hT8_new[:, k, :, :], hp8[:].bitcast(BF16)
                            )
                        del pair_state[k]
                # For pair 1 (w==3) the bf16 DMAT queues FIRST: the decode
                # dense (critical chain: dense->pt->x) waits on pair1-bf16,
                # while pair1-fp8's consumers sit mid-next-step with slack.
                # Pair 0 keeps fp8-first (it feeds the next step's first DR).
                if fp8_dmat is not None and w != 3:
                    fp8_dmat(); fp8_dmat = None
                if hT_new is not None:
                    # Pair-merged bf16 transpose: ONE DMAT per wave pair
                    # (w0,w1)/(w2,w3) instead of one per wave — the xbar
                    # serializes transposes at ~1.3us each, and decode issues
                    # fp8-pair + bf16 transposes back-to-back. Slot = w%2 so
                    # the container is unit-ascending -> chunk-major dst.
                    k, sb = PAIRK[w], w % 2
                    if k not in pairb_state:
                        pairb_state[k] = (
                            gpool.tile([128, 2, QW], BF16, tag="hbf", name="hbf"),
                            set(),
                        )
                    hbf, written = pairb_state[k]
                    dve_insts[("hb", w)] = nc.vector.tensor_mul(
                        hbf[:, sb, :], sig[:, 2 * QW : 3 * QW], tcc[:]
                    )
                    written.add(w)
                    if len(written) == 2:
                        nc.sync.dma_start_transpose(
                            hT_new[:, 4 * k : 4 * (k + 1), :], hbf[:]
                        )
                        del pairb_state[k]
                if fp8_dmat is not None:
                    fp8_dmat()

            def emit_gates(z, w, hT_new, hT8_new, merged=False):
                """Unsplit gate math (bf16 steps): gates_a + gates_b."""
                sig = gates_a(z, w)
                gates_b(sig, w, hT_new, hT8_new)

            # fp8 pair-pack layout: wave w's h lands in pair k = PAIRK[w] at
            # byte slot SLOT[w]; the pair is transposed after its second
            # writer in gate-B order [1, 0, 2, 3].
            PAIRK = {0: 0, 1: 0, 2: 1, 3: 1}
            SLOT = {1: 0, 0: 1, 2: 0, 3: 1}
            pair_state = {}
            pairb_state = {}

            def h8chunk(hT8, ch):
                """DoubleRow lhsT AP [128, 2, 128] for chunk ch (units of
                wave ch) out of the pair-transposed container."""
                v = hT8[:, PAIRK[ch], :, :].bitcast(FP8).rearrange(
                    "p a (m s) -> p a m s", s=2
                )
                return v[:, :, :, SLOT[ch]]

            def emit_step_fp8(x_lhsT, hT8_prev, fp8_out, bf16_also=False):
                """One fp8 LSTM step (DoubleRow h matmuls).

                The PE order is PINNED with sync=False deps (the Tile
                scheduler otherwise reorders all ch2/ch3 sections to the
                stream tail, which pushes z1's close — and with it the whole
                sig->DVE->tanh->h->transpose produce chain — 2.4us later,
                inflating the steady-state period). Pinning closes wave 1 at
                ~MM 16 of 40: the pair-0 chain starts mid-stream and the
                next step's first DR consumption meets it with ~0 idle.
                """
                hT_new = (
                    hpool.tile([128, NU, 128], BF16, tag="hT", name="hT_new")
                    if (bf16_also or not fp8_out)
                    else None
                )
                hT8_new = (
                    hpool8.tile([128, 2, 2, 128], BF16, tag="hT8", name="hT8_new")
                    if fp8_out
                    else None
                )
                zs = [
                    zpool.tile([128, WW], F32, tag="z", name=f"z{w}") for w in range(NW)
                ]
                x_only = hT8_prev is None
                pe = []

                def xsec(w, stop=False):
                    for n in range(NB):
                        pe.append(nc.tensor.matmul(
                            zs[w][:, n * 512 : (n + 1) * 512],
                            x_lhsT,
                            Wk_sb[:, WW * w + n * 512 : WW * w + (n + 1) * 512],
                            start=True,
                            stop=stop,
                        ))

                A = {}
                def close(w):
                    A[w] = gates_a(zs[w], w)
                def flush(w):
                    gates_b(A.pop(w), w, hT_new, hT8_new)

                if x_only:
                    for w in (1, 0, 2, 3):
                        xsec(w, stop=True)
                    close(1); close(0); flush(1); flush(0)
                    close(2); close(3); flush(2); flush(3)
                    return hT_new, hT8_new

                def dr(w, ch, stop=False):
                    for n in range(NB):
                        pe.append(nc.tensor.matmul(
                            zs[w][:, n * 512 : (n + 1) * 512],
                            h8chunk(hT8_prev, ch),
                            Wr8_sb[:, ch, :, WW * w + n * 512 : WW * w + (n + 1) * 512],
                            start=False,
                            stop=stop,
                            perf_mode=DR,
                        ))

                xsec(1); xsec(0)
                # PE-warming filler: duplicate x-matmuls (start=True re-clears
                # and rewrites the same contribution — idempotent since the DR
                # accumulates come later). The ~2.5us pair-0 wait otherwise
                # re-throttles HAM to 1.2GHz, making the first ~2.6us of DR
                # matmuls run at half clock every step.
                for _ in range(2):
                    xsec(1); xsec(0)   # PE-warming duplicate rounds
                dr(1, 0); dr(1, 1); dr(0, 0); dr(0, 1)
                xsec(2); dr(2, 0); dr(2, 1)
                dr(1, 2); dr(1, 3, stop=True); close(1)
                dr(0, 2); dr(0, 3, stop=True); close(0)
                xsec(3); dr(3, 0); dr(3, 1)
                dr(2, 2); dr(2, 3, stop=True); close(2)
                flush(1); flush(0)
                dr(3, 2); dr(3, 3, stop=True); close(3)
                flush(2); flush(3)
                for a, b in zip(pe, pe[1:]):
                    add_dep_helper(b.ins, a.ins, sync=False, reason="pinned PE order")
                pin_scalar([("sig", 1), ("sig", 0), ("tcc", 1), ("tcc", 0),
                            ("sig", 2), ("sig", 3), ("tcc", 2), ("tcc", 3)])
                pin_dve([("t1", 1), ("t2", 1), ("c", 1),
                         ("t1", 0), ("t2", 0), ("c", 0),
                         ("h", 1), ("hb", 1), ("h", 0), ("hb", 0),
                         ("t1", 2), ("t2", 2), ("c", 2),
                         ("t1", 3), ("t2", 3), ("c", 3),
                         ("h", 2), ("hb", 2), ("h", 3), ("hb", 3)])
                return hT_new, hT8_new

            def emit_step(x_lhsT, hT_prev, x_first):
                """One bf16 LSTM step; returns (hT_new bf16, None)."""
                hT_new = hpool.tile([128, NU, 128], BF16, tag="hT", name="hT_new")
                hT8_new = None
                for w in range(NW):
                    base = WW * w
                    z = zpool.tile([128, WW], F32, tag="z", name="z")
                    ks = []
                    if x_first:
                        ks.append(("x", 0))
                    if hT_prev is not None:
                        ks += [("h", u) for u in range(NU)]
                    if not x_first:
                        ks.append(("x", 0))
                    # k-outer / n-inner: the first-emitted matmuls depend on
                    # operands ready earliest (x, then low h chunks), so the
                    # PE can start the next step while the previous step's
                    # late h chunks are still in flight through the
                    # gate-math chain.
                    for ki, (kind, kv) in enumerate(ks):
                        st, sp = ki == 0, ki == len(ks) - 1
                        for n in range(NB):
                            zsl = z[:, n * 512 : (n + 1) * 512]
                            cs = slice(base + n * 512, base + (n + 1) * 512)
                            if kind == "x":
                                nc.tensor.matmul(
                                    zsl, x_lhsT, Wk_sb[:, cs], start=st, stop=sp
                                )
                            else:
                                nc.tensor.matmul(
                                    zsl,
                                    hT_prev[:, kv, :],
                                    Wr_sb[:, kv, cs],
                                    start=st,
                                    stop=sp,
                                )
                    emit_gates(z, w, hT_new, hT8_new, merged=False)
                return hT_new, hT8_new

            def emit_dense(hT_cur, out_idx, feedback, zp=None, pe=None):
                if zp is None:
                    zp = zpool.tile([128, WW], F32, tag="z", name="zdense")
                pp = zp[0:I, 0:BC]
                for u in range(NU):
                    mm = nc.tensor.matmul(
                        pp,
                        Wd_sb[:, u, :],
                        hT_cur[:, u, :],
                        start=(u == 0),
                        stop=(u == NU - 1),
                    )
                    if pe is not None:
                        pe.append(mm)
                if feedback:
                    pt = gpool.tile([KX, BC], BF16, tag="pT", name="pT")
                    nc.gpsimd.memset(pt[I : I + 1, :], 1.0)
                    act_insts[("pt", 0)] = nc.scalar.activation(
                        pt[0:I, :], pp, AF.Identity, bias=bd_sb[:]
                    )
                else:
                    pt = None
                pf = gpool.tile([I, BC], F32, tag="pf", name="pf")
                act_insts[("pf", 0)] = nc.scalar.activation(
                    pf[:], pp, AF.Identity, bias=bd_sb[:]
                )
                nc.scalar.dma_start(outD[out_idx], pf[:])
                return pt

            def emit_step_dec(hT_prev, out_idx):
                """Decode step: consumes hT_prev for BOTH the recurrent
                matmuls and the dense head of the PREVIOUS step's output
                (out_idx), whose result pt feeds this step's x-part. The
                dense matmuls run mid-stream (after wave 1's h-section,
                when hT_prev's late chunks have landed) and all x-matmuls
                come after pt, so the PE never idles waiting for the
                h-transpose tail + dense chain between steps."""
                hT_new = hpool.tile([128, NU, 128], BF16, tag="hT", name="hT_new")
                zp = zpool.tile([128, WW], F32, tag="z", name="zdense")
                zs = [
                    zpool.tile([128, WW], F32, tag="z", name=f"z{w}") for w in range(NW)
                ]

                def hsec(w):
                    for u in range(NU):
                        for n in range(NB):
                            nc.tensor.matmul(
                                zs[w][:, n * 512 : (n + 1) * 512],
                                hT_prev[:, u, :],
                                Wr_sb[:, u, WW * w + n * 512 : WW * w + (n + 1) * 512],
                                start=(u == 0),
                                stop=False,
                            )

                def xsec(w):
                    for n in range(NB):
                        nc.tensor.matmul(
                            zs[w][:, n * 512 : (n + 1) * 512],
                            pt[:],
                            Wk_sb[:, WW * w + n * 512 : WW * w + (n + 1) * 512],
                            start=False,
                            stop=True,
                        )

                hsec(0)
                hsec(1)
                pt = emit_dense(hT_prev, out_idx, feedback=True, zp=zp)
                xsec(0)
                emit_gates(zs[0], 0, hT_new, None)
                xsec(1)
                emit_gates(zs[1], 1, hT_new, None)
                for w in (2, 3):
                    hsec(w)
                    xsec(w)
                    emit_gates(zs[w], w, hT_new, None)
                return hT_new

            def emit_step_dec8(hT_prev, hT8_prev, out_idx):
                """fp8 decode step: DR recurrent matmuls (fp8 pairs) + bf16
                dense head (reads hT_prev; fp8 h x fp8 Wd straight to the
                output measured rel-err 0.032 > gate) of the PREVIOUS step's
                output (out_idx); pt feeds this step's x-part, which closes
                each wave (start comes from dr(w,0))."""
                hT_new = hpool.tile([128, NU, 128], BF16, tag="hT", name="hT_new")
                hT8_new = hpool8.tile([128, 2, 2, 128], BF16, tag="hT8", name="hT8_new")
                # No 5th PSUM tile for the dense head (4 z tiles = all 8
                # banks): pp borrows zs[3]'s bank n0. dr(3,0)'s start=True
                # reclears it after the pt/pf reads (WAR dep), so wave 3's
                # sections are emitted after the dense.
                zs = [
                    zpool.tile([128, WW], F32, tag="z", name=f"z{w}") for w in range(NW)
                ]

                pe = []

                def dr(w, ch, start=False, stop=False):
                    for n in range(NB):
                        pe.append(nc.tensor.matmul(
                            zs[w][:, n * 512 : (n + 1) * 512],
                            h8chunk(hT8_prev, ch),
                            Wr8_sb[:, ch, :, WW * w + n * 512 : WW * w + (n + 1) * 512],
                            start=start,
                            stop=stop,
                            perf_mode=DR,
                        ))

                def xse(w):
                    for n in range(NB):
                        pe.append(nc.tensor.matmul(
                            zs[w][:, n * 512 : (n + 1) * 512],
                            pt[:],
                            Wk_sb[:, WW * w + n * 512 : WW * w + (n + 1) * 512],
                            start=False,
                            stop=True,
                        ))

                A = {}
                def close(w):
                    A[w] = gates_a(zs[w], w)
                def flush(w):
                    gates_b(A.pop(w), w, hT_new, hT8_new)

                # Pinned PE order (see emit_step_fp8): pair0 chunks of waves
                # 1,0,2 first, then pair1 chunks of w1, then dense (needs all
                # of hT_prev) + pt, then x closes each wave; wave 3 entirely
                # after the dense (its banks host pp).
                dr(1, 0, start=True); dr(1, 1); dr(0, 0, start=True); dr(0, 1)
                dr(2, 0, start=True); dr(2, 1)
                dr(1, 2); dr(1, 3)
                pt = emit_dense(hT_prev, out_idx, feedback=True, zp=zs[3],
                                pe=pe)
                xse(1); close(1)
                dr(0, 2); dr(0, 3); xse(0); close(0)
                dr(2, 2); dr(2, 3); xse(2); close(2)
                flush(1); flush(0)
                dr(3, 0, start=True); dr(3, 1); dr(3, 2); dr(3, 3); xse(3)
                close(3); flush(2); flush(3)
                # PE-warming filler: replay wave 3's DR sections into z3 after
                # sig3's read (WAR-ordered; nothing reads the replay — the
                # next step's dr(3,0) start=True reclears). Covers most of the
                # ~3.2us pair-0 wait so HAM stays at full clock.
                dr(3, 0, start=True); dr(3, 1); dr(3, 2); dr(3, 3, stop=True)
                for a, b in zip(pe, pe[1:]):
                    add_dep_helper(b.ins, a.ins, sync=False, reason="pinned PE order")
                pin_scalar([("pt", 0), ("sig", 1), ("sig", 0), ("pf", 0),
                            ("tcc", 1), ("tcc", 0), ("sig", 2), ("sig", 3),
                            ("tcc", 2), ("tcc", 3)])
                pin_dve([("t1", 1), ("t2", 1), ("c", 1),
                         ("t1", 0), ("t2", 0), ("c", 0),
                         ("h", 1), ("hb", 1), ("h", 0), ("hb", 0),
                         ("t1", 2), ("t2", 2), ("c", 2),
                         ("t1", 3), ("t2", 3), ("c", 3),
                         ("h", 2), ("hb", 2), ("h", 3), ("hb", 3)])
                return hT_new, hT8_new

            hT = hT8 = None
            # prefetch input blocks one block (XBLK steps) ahead so the
            # stream DMA never sits on the first x-matmul's critical path
            nblk_used = (n_warm + XBLK - 1) // XBLK
            xtiles = {}
            if nblk_used > 0:
                xtiles[0] = xpool.tile([KX, XBLK * BC], BF16, tag="xblk", name="xblk")
                nc.sync.dma_start(xtiles[0][:], xTbD[0])
            for t in range(n_warm):
                b = t // XBLK
                s = t % XBLK
                f_out = is_fp8(t + 1) if t + 1 < n_steps else False
                xl = xtiles[b][:, s * BC : (s + 1) * BC]
                if is_fp8(t):
                    both = DECODE_FP8 and n_dec > 0 and t == n_warm - 1
                    hT, hT8 = emit_step_fp8(xl, hT8, f_out, bf16_also=both)
                else:
                    hT, hT8 = emit_step(xl, hT, x_first=True)
                if t % XBLK == 0 and b + 1 < nblk_used:
                    # prefetch the next input block; emitted after the step so
                    # it queues behind this step's critical hT transposes
                    xtiles[b + 1] = xpool.tile([KX, XBLK * BC], BF16, tag="xblk", name="xblk")
                    nc.sync.dma_start(xtiles[b + 1][:], xTbD[b + 1])
                xtiles.pop(b - 1, None)
            if n_dec == 0:
                emit_dense(hT, 0, feedback=False)
            elif DECODE_FP8:
                for s_ in range(1, n_dec + 1):
                    hT, hT8 = emit_step_dec8(hT, hT8, s_ - 1)
                emit_dense(hT, n_dec, feedback=False)
            else:
                for s_ in range(1, n_dec + 1):
                    hT = emit_step_dec(hT, s_ - 1)
                emit_dense(hT, n_dec, feedback=False)

    nc.finalize()
    return nc


def prep_in_maps(inputs, Wk, Wr, b, Wd, bd, n_warm=T):
    """Host-side sharding + layout. inputs [B, T, I] fp32; returns 8 in_maps."""
    perm = _gate_perm()
    cs = SC * _gate_colscale()                                 # 2*SC on g columns
    Wk_aug = np.concatenate(
        [np.asarray(Wk, np.float32), np.asarray(b, np.float32)[None, :]], axis=0
    )
    Wk_p = (Wk_aug[:, perm] * cs).astype(NPBF16)               # [65, 4096] scaled
    Wr_s = np.asarray(Wr, np.float32)[:, perm] * cs            # [1024, 4096] scaled
    Wr_p = Wr_s.reshape(NU, 128, 4 * U).transpose(1, 0, 2).astype(NPBF16).copy()
    # DoubleRow layout: [p, ch, j, n] = Wr_s[ch*256 + j*128 + p, n]
    Wr_8 = (
        Wr_s.reshape(NCH, 2, 128, 4 * U).transpose(2, 0, 1, 3).astype(NPFP8).copy()
    )
    Wd_f = np.asarray(Wd, np.float32).reshape(NU, 128, I).transpose(1, 0, 2)
    Wd_p = Wd_f.astype(NPBF16).copy()
    Wd_8 = (Wd_f * SC).astype(NPFP8).copy()
    bd_c = np.asarray(bd, np.float32).reshape(I, 1).copy()

    x = np.asarray(inputs, np.float32)
    nblk = (n_warm + XBLK - 1) // XBLK
    in_maps = []
    for c in range(NCORES):
        xc = x[c * BC : (c + 1) * BC, :n_warm]                 # [BC, n_warm, I]
        xT = np.transpose(xc, (1, 2, 0))                       # [n_warm, I, BC]
        xTa = np.concatenate([xT, np.ones((n_warm, 1, BC), np.float32)], axis=1)
        if nblk * XBLK != n_warm:
            pad = np.zeros((nblk * XBLK - n_warm, KX, BC), np.float32)
            xTa = np.concatenate([xTa, pad], axis=0)
        xTb = (
            xTa.reshape(nblk, XBLK, KX, BC)
            .transpose(0, 2, 1, 3)
            .reshape(nblk, KX, XBLK * BC)
            .astype(NPBF16)
            .copy()
        )
        in_maps.append(
            {"xTb": xTb, "Wk": Wk_p, "Wr": Wr_p, "Wr8": Wr_8, "Wd": Wd_p,
             "Wd8": Wd_8, "bdc": bd_c}
        )
    return in_maps


_NC_CACHE = {}


def _get_nc(n_warm, n_dec):
    key = (n_warm, n_dec)
    if key not in _NC_CACHE:
        _NC_CACHE[key] = build_nc(n_warm, n_dec)
    return _NC_CACHE[key]


def run(inputs, Wk, Wr, b, Wd, bd, n_warm, n_dec, trace=False, **kw):
    nc = _get_nc(n_warm, n_dec)
    in_maps = prep_in_maps(inputs, Wk, Wr, b, Wd, bd, n_warm)
    res = run_bass_kernel_spmd(nc, in_maps, list(range(NCORES)), trace=trace, **kw)
    outs = [np.asarray(res.results[c]["out"], np.float32) for c in range(NCORES)]
    # out[c]: [n_dec+1, I, BC] -> preds [B, n_dec+1, I]
    preds = np.concatenate([o.transpose(2, 0, 1) for o in outs], axis=0)
    return preds, res


def kernel(inputs, Wk, Wr, b, Wd, bd, output_indices, output_steps):
    n_dec = int(output_steps) - 1
    preds, _ = run(inputs, Wk, Wr, b, Wd, bd, T, n_dec)
    idx = np.asarray(output_indices, np.int64)
    return np.take(preds, idx, axis=-1).astype(np.float32)

